# revision 1
# baseline (speedup 1.0000x reference)
"""RNN-T JointNetwork kernel for 8 Trainium2 NeuronCores.

Math: out[b,t,u,:] = tanh(concat(fe[b,t], gd[b,u])) @ Wj + bj
with fe = f@We+be, gd = g@Wd+bd.

Since tanh acts elementwise and the concat feeds a single GEMM, the joint
GEMM factorizes exactly:
    out[b,t,u,:] = A[b,t,:] + C[b,u,:]
    A = tanh(f@We+be) @ Wj[:Dm]          (per-(b,t) row)
    C = tanh(g@Wd+bd) @ Wj[Dm:] + bj     (per-(b,u) row)
This collapses the 137-GFLOP joint GEMM into two tiny GEMMs plus a
broadcast-add, leaving the kernel bound by the 268 MB output write.

Sharding: 8 cores, core c owns (b = c//2, t-half = c%2) -> a [128,64,V]
output chunk per core (contiguous 33.5 MB).

On-core plan (all fp32):
  - fe^T[m,t] = (We.T @ f^T) via PE (f^T from PE transpose), tanh+bias on ACT
  - gd^T[m,u] likewise
  - A[t,v] (psum = tfT.T @ Wj_top), Cp[u,v] (tgT.T @ Wj_bot + 1x bj)
  - Crep[0:128,v] = Cp stacked twice (selector matmul)
  - per 128-row output tile k (= t-pair 2k,2k+1): broadcast A rows with a
    constant 32-row selector bank (32-aligned slices of A as matmul rhs),
    add Crep on DVE (half 0) / replicate Cp on PE + copy on ACT (half 1),
    DMA 512 KB contiguous per tile.
"""

import sys

sys.path.insert(0, "/opt/trn_rl_repo")

import numpy as np

import concourse.bacc as bacc
import concourse.mybir as mybir
import concourse.tile as tile
from concourse.bass_utils import run_bass_kernel_spmd
from concourse.masks import make_identity

B, T, U = 4, 256, 64
D = 512  # DE = DD = DM
V = 1024
TC = 128  # t rows per core
NCORES = 8
FP32 = mybir.dt.float32
BF16 = mybir.dt.bfloat16
TANH = mybir.ActivationFunctionType.Tanh

_cache = {}


def _build_nc():
    nc = bacc.Bacc("TRN2", target_bir_lowering=False)

    f_d = nc.dram_tensor("f_c", [TC, D], FP32, kind="ExternalInput")
    g_d = nc.dram_tensor("g_c", [U, D], FP32, kind="ExternalInput")
    We_d = nc.dram_tensor("We", [D, D], FP32, kind="ExternalInput")
    be_d = nc.dram_tensor("be", [D], FP32, kind="ExternalInput")
    Wd_d = nc.dram_tensor("Wd", [D, D], FP32, kind="ExternalInput")
    bd_d = nc.dram_tensor("bd", [D], FP32, kind="ExternalInput")
    Wj_d = nc.dram_tensor("Wj", [2 * D, V], FP32, kind="ExternalInput")
    bj_d = nc.dram_tensor("bj", [V], FP32, kind="ExternalInput")
    out_d = nc.dram_tensor("out", [TC * U, V], FP32, kind="ExternalOutput")

    with tile.TileContext(nc) as tc:
        with (
            tc.tile_pool(name="const", bufs=1) as cp,
            tc.tile_pool(name="wts", bufs=1) as wp,
        ):
            # ---- constants ----
            ident = cp.tile([128, 128], FP32, tag="ident")
            make_identity(nc, ident[:])

            # selrep[u, j] = 1 iff j%64 == u  ([I64 | I64])
            selrep = cp.tile([64, 128], FP32, tag="selrep")
            nc.gpsimd.memset(selrep[:], 0.0)
            nc.gpsimd.affine_select(
                out=selrep[:].rearrange("p (a b) -> p a b", a=2),
                in_=selrep[:].rearrange("p (a b) -> p a b", a=2),
                compare_op=mybir.AluOpType.not_equal,
                fill=1.0,
                base=0,
                pattern=[[0, 2], [-1, 64]],
                channel_multiplier=1,
            )

            # sel32[32q + t', 128i + 64jh + jl] = 1 iff t' == 2i + jh
            # (identical pattern in each 32-partition strip q). bf16: the
            # selector is 0/1 so bf16 matmuls against bf16 hi/lo terms of A
            # select exactly, at 1 col/cycle instead of fp32's multi-pass.
            sel32 = cp.tile([128, 16 * 128], BF16, tag="sel32")
            nc.gpsimd.memset(sel32[:], 0.0)
            for q in range(4):
                sl = sel32[32 * q : 32 * q + 32, :]
                nc.gpsimd.affine_select(
                    out=sl.rearrange("p (i a b) -> p i a b", i=16, a=2),
                    in_=sl.rearrange("p (i a b) -> p i a b", i=16, a=2),
                    compare_op=mybir.AluOpType.not_equal,
                    fill=1.0,
                    base=0,
                    pattern=[[-2, 16], [-1, 2], [0, 64]],
                    channel_multiplier=1,
                )

            ones1 = cp.tile([1, 64], FP32, tag="ones1")
            nc.gpsimd.memset(ones1[:], 1.0)

            # dup_hi/dup_lo: build AHL = [Ahi(0:32); Alo(0:32); Ahi(32:64);
            # Alo(32:64)] per 64-row half. j = 64*jh2 + 32*jm + jl.
            # dup_hi[t', j] = 1 iff jm==0 and t' == 32*jh2 + jl
            # dup_lo[t', j] = 1 iff jm==1 and t' == 32*jh2 + jl
            dup_hi = cp.tile([128, 128], BF16, tag="dup_hi")
            dup_lo = cp.tile([128, 128], BF16, tag="dup_lo")
            for tile_, base in ((dup_hi, 0), (dup_lo, 64)):
                nc.gpsimd.memset(tile_[:], 0.0)
                for s in range(2):
                    sl = tile_[64 * s : 64 * s + 64, :]
                    nc.gpsimd.affine_select(
                        out=sl.rearrange("p (a b c) -> p a b c", a=2, b=2),
                        in_=sl.rearrange("p (a b c) -> p a b c", a=2, b=2),
                        compare_op=mybir.AluOpType.not_equal,
                        fill=1.0,
                        base=base,
                        pattern=[[-32, 2], [-64 if base else 64, 2], [-1, 32]],
                        channel_multiplier=1,
                    )

            # ---- persistent operands ----
            f_sb = wp.tile([TC, D], FP32, tag="f")
            g_sb = wp.tile([U, D], FP32, tag="g")
            We_sb = [wp.tile([128, D], FP32, tag=f"We{c}", name=f"We{c}") for c in range(4)]
            Wd_sb = [wp.tile([128, D], FP32, tag=f"Wd{c}", name=f"Wd{c}") for c in range(4)]
            Wj_sb = [wp.tile([128, V], FP32, tag=f"Wj{c}", name=f"Wj{c}") for c in range(8)]
            be_sb = [wp.tile([128, 1], FP32, tag=f"be{c}", name=f"be{c}") for c in range(4)]
            bd_sb = [wp.tile([128, 1], FP32, tag=f"bd{c}", name=f"bd{c}") for c in range(4)]
            bj_sb = wp.tile([1, V], FP32, tag="bj")
            fT = [wp.tile([128, TC], FP32, tag=f"fT{c}", name=f"fT{c}") for c in range(4)]
            gT = [wp.tile([128, U], FP32, tag=f"gT{c}", name=f"gT{c}") for c in range(4)]
            tfT = [wp.tile([128, TC], FP32, tag=f"tfT{c}", name=f"tfT{c}") for c in range(4)]
            tgT = [wp.tile([128, U], FP32, tag=f"tgT{c}", name=f"tgT{c}") for c in range(4)]
            A_sb = wp.tile([TC, V], FP32, tag="A")
            A_hi = wp.tile([TC, V], BF16, tag="A_hi")
            A_lo = wp.tile([TC, V], BF16, tag="A_lo")
            A_tmp = wp.tile([TC, V], FP32, tag="A_tmp")
            AHL = [wp.tile([128, V], BF16, tag=f"AHL{h}", name=f"AHL{h}") for h in range(2)]
            Cp = wp.tile([U, V], FP32, tag="Cp")
            Crep = wp.tile([128, V], FP32, tag="Crep")

            nc.sync.dma_start(f_sb[:], f_d[:])
            nc.sync.dma_start(g_sb[:], g_d[:])
            for c in range(4):
                nc.sync.dma_start(We_sb[c][:], We_d[c * 128 : (c + 1) * 128, :])
                nc.sync.dma_start(Wd_sb[c][:], Wd_d[c * 128 : (c + 1) * 128, :])
                nc.sync.dma_start(
                    be_sb[c][:],
                    be_d[c * 128 : (c + 1) * 128].rearrange("(p o) -> p o", o=1),
                )
                nc.sync.dma_start(
                    bd_sb[c][:],
                    bd_d[c * 128 : (c + 1) * 128].rearrange("(p o) -> p o", o=1),
                )
            for c in range(8):
                nc.sync.dma_start(Wj_sb[c][:], Wj_d[c * 128 : (c + 1) * 128, :])
            nc.sync.dma_start(bj_sb[:], bj_d.rearrange("(p v) -> p v", p=1))

            # ---- prologue: A, Cp, Crep ----
            with tc.tile_pool(name="pp", bufs=4, space="PSUM") as pp:
                for c in range(4):
                    pt = pp.tile([128, 128], FP32, tag="pps")
                    nc.tensor.transpose(
                        pt[:], f_sb[:, c * 128 : (c + 1) * 128], ident[:]
                    )
                    nc.vector.tensor_copy(fT[c][:], pt[:])
                for c in range(4):
                    pt = pp.tile([128, U], FP32, tag="pps")
                    nc.tensor.transpose(
                        pt[:], g_sb[:, c * 128 : (c + 1) * 128], ident[0:64, 0:64]
                    )
                    nc.vector.tensor_copy(gT[c][:], pt[:])

                for mc in range(4):
                    ms = slice(mc * 128, (mc + 1) * 128)
                    ps = pp.tile([128, TC], FP32, tag="pps")
                    for dc in range(4):
                        nc.tensor.matmul(
                            ps[:],
                            We_sb[dc][:, ms],
                            fT[dc][:],
                            start=(dc == 0),
                            stop=(dc == 3),
                        )
                    nc.scalar.activation(
                        tfT[mc][:], ps[:], TANH, bias=be_sb[mc][:, 0:1]
                    )
                for mc in range(4):
                    ms = slice(mc * 128, (mc + 1) * 128)
                    ps = pp.tile([128, U], FP32, tag="pps")
                    for dc in range(4):
                        nc.tensor.matmul(
                            ps[:],
                            Wd_sb[dc][:, ms],
                            gT[dc][:],
                            start=(dc == 0),
                            stop=(dc == 3),
                        )
                    nc.scalar.activation(
                        tgT[mc][:], ps[:], TANH, bias=bd_sb[mc][:, 0:1]
                    )

                for vh in range(2):
                    vs = slice(vh * 512, (vh + 1) * 512)
                    ps = pp.tile([128, 512], FP32, tag="pps")
                    for mc in range(4):
                        nc.tensor.matmul(
                            ps[:],
                            tfT[mc][:],
                            Wj_sb[mc][:, vs],
                            start=(mc == 0),
                            stop=(mc == 3),
                        )
                    nc.vector.tensor_copy(A_sb[:, vs], ps[:])
                for vh in range(2):
                    vs = slice(vh * 512, (vh + 1) * 512)
                    ps = pp.tile([64, 512], FP32, tag="pps")
                    for mc in range(4):
                        nc.tensor.matmul(
                            ps[:],
                            tgT[mc][:],
                            Wj_sb[4 + mc][:, vs],
                            start=(mc == 0),
                            stop=False,
                        )
                    nc.tensor.matmul(
                        ps[:], ones1[:], bj_sb[:, vs], start=False, stop=True
                    )
                    nc.scalar.copy(Cp[:, vs], ps[:])
                for vh in range(2):
                    vs = slice(vh * 512, (vh + 1) * 512)
                    ps = pp.tile([128, 512], FP32, tag="pps")
                    nc.tensor.matmul(ps[:], selrep[:], Cp[:, vs], start=True, stop=True)
                    nc.vector.tensor_copy(Crep[:, vs], ps[:])

                # exact-ish two-term bf16 split A = A_hi + A_lo + O(2^-17),
                # done per 64-row half so AHL[0] (tiles 0..31) unblocks early;
                # AHL[h] = [Ahi(64h+0:32); Alo(same); Ahi(64h+32:64); Alo(same)]
                # via dup-selector matmuls (bf16 0/1 select, exact)
                for h in range(2):
                    hs = slice(64 * h, 64 * h + 64)
                    nc.vector.tensor_copy(A_hi[hs, :], A_sb[hs, :])
                    nc.vector.tensor_copy(A_tmp[hs, :], A_hi[hs, :])
                    nc.vector.tensor_sub(A_tmp[hs, :], A_sb[hs, :], A_tmp[hs, :])
                    nc.vector.tensor_copy(A_lo[hs, :], A_tmp[hs, :])
                    for vh in range(2):
                        vs = slice(vh * 512, (vh + 1) * 512)
                        ps = pp.tile([128, 512], FP32, tag="pps")
                        nc.tensor.matmul(
                            ps[:], dup_hi[hs, :], A_hi[hs, vs],
                            start=True, stop=False, tile_position=(64 * h, 0),
                        )
                        nc.tensor.matmul(
                            ps[:], dup_lo[hs, :], A_lo[hs, vs],
                            start=False, stop=True, tile_position=(64 * h, 0),
                        )
                        nc.vector.tensor_copy(AHL[h][:, vs], ps[:])

            # ---- main loop: 64 output tiles of [128, 1024] ----
            with (
                tc.tile_pool(name="po", bufs=4, space="PSUM") as po,
                tc.tile_pool(name="ob", bufs=8) as ob,
            ):
                for k in range(64):
                    q, i = k // 16, k % 16
                    h, r = q // 2, q % 2
                    rs = slice(64 * r, 64 * r + 64)
                    lhs_sel = sel32[rs, i * 128 : (i + 1) * 128]
                    psO = po.tile([128, V], FP32, tag="psO")
                    out_sb = ob.tile([128, V], FP32, tag="out")
                    # A broadcast (hi+lo packed, K=64) on PE, one MM per bank
                    for vh in range(2):
                        vs = slice(vh * 512, (vh + 1) * 512)
                        nc.tensor.matmul(
                            psO[:, vs], lhs_sel, AHL[h][rs, vs],
                            start=True, stop=True, tile_position=(64 * r, 0),
                        )
                    # single full-width DVE add does C + the PSUM->SBUF move
                    nc.vector.tensor_add(out_sb[:], psO[:], Crep[:])
                    nc.sync.dma_start(
                        out_d[k * 128 : (k + 1) * 128, :], out_sb[:]
                    )

    nc.compile()
    return nc


def kernel(f, g, We, be, Wd, bd, Wj, bj):
    if "nc" not in _cache:
        _cache["nc"] = _build_nc()
    nc = _cache["nc"]

    cast = lambda x: np.ascontiguousarray(np.asarray(x), dtype=np.float32)
    f, g = cast(f), cast(g)
    shared = {
        "We": cast(We), "be": cast(be), "Wd": cast(Wd), "bd": cast(bd),
        "Wj": cast(Wj), "bj": cast(bj),
    }
    in_maps = []
    for c in range(NCORES):
        b, th = c // 2, c % 2
        in_maps.append(
            {
                "f_c": np.ascontiguousarray(f[b, th * TC : (th + 1) * TC, :]),
                "g_c": np.ascontiguousarray(g[b]),
                **shared,
            }
        )
    res = run_bass_kernel_spmd(nc, in_maps, list(range(NCORES)))
    kernel._last_results = res

    out = np.empty((B, T, U, V), np.float32)
    for c in range(NCORES):
        b, th = c // 2, c % 2
        out[b, th * TC : (th + 1) * TC] = res.results[c]["out"].reshape(TC, U, V)
    return out



# revision 3
# speedup vs baseline: 1.1008x; 1.1008x over previous
"""RNN-T JointNetwork kernel for 8 Trainium2 NeuronCores.

Math: out[b,t,u,:] = tanh(concat(fe[b,t], gd[b,u])) @ Wj + bj
with fe = f@We+be, gd = g@Wd+bd.

Since tanh acts elementwise and the concat feeds a single GEMM, the joint
GEMM factorizes exactly:
    out[b,t,u,:] = A[b,t,:] + C[b,u,:]
    A = tanh(f@We+be) @ Wj[:Dm]          (per-(b,t) row)
    C = tanh(g@Wd+bd) @ Wj[Dm:] + bj     (per-(b,u) row)
This collapses the 137-GFLOP joint GEMM into two tiny GEMMs plus a
broadcast-add, leaving the kernel bound by the 268 MB output write.

Sharding: 8 cores, core c owns (b = c//2, t-half = c%2) -> a [128,64,V]
output chunk per core (contiguous 33.5 MB).

On-core plan (bf16 weights/activations, fp32 output; tolerance 2e-2 vs
bf16's ~5e-3 makes this safe):
  - host pre-casts We/Wd/Wj/bj to bf16 and pre-transposes f,g -> fT,gT
    so the device reads 3.2 MB instead of 6.7 MB
  - tfT[m,t] = tanh(We.T@fT + be), tgT likewise (PE bf16 + ACT tanh)
  - ACp0 = [A(0:64) ; C] and ACp1 = [C ; A(64:128)] packed bf16
    [128,V] tiles (C carries the bj bias, added via a K=1 ones matmul)
  - per 128-row output tile k (t-pair 2k,2k+1): ONE K=128 selector
    matmul per 512-col bank picks the A row and the C row and sums them
    in fp32 PSUM; a plain PSUM->SBUF copy (alternating DVE/ACT) then a
    1 MB DMA per pair of tiles
"""

import sys

sys.path.insert(0, "/opt/trn_rl_repo")

import numpy as np

import concourse.bacc as bacc
import concourse.mybir as mybir
import concourse.tile as tile
from concourse.bass_utils import run_bass_kernel_spmd

B, T, U = 4, 256, 64
D = 512  # DE = DD = DM
V = 1024
TC = 128  # t rows per core
NCORES = 8
FP32 = mybir.dt.float32
BF16 = mybir.dt.bfloat16
NPBF16 = mybir.dt.np(mybir.dt.bfloat16)
TANH = mybir.ActivationFunctionType.Tanh

_cache = {}


def _build_nc():
    nc = bacc.Bacc("TRN2", target_bir_lowering=False)

    fT_d = nc.dram_tensor("fT_c", [D, TC], BF16, kind="ExternalInput")
    gT_d = nc.dram_tensor("gT_c", [D, U], BF16, kind="ExternalInput")
    We_d = nc.dram_tensor("We", [D, D], BF16, kind="ExternalInput")
    be_d = nc.dram_tensor("be", [D], FP32, kind="ExternalInput")
    Wd_d = nc.dram_tensor("Wd", [D, D], BF16, kind="ExternalInput")
    bd_d = nc.dram_tensor("bd", [D], FP32, kind="ExternalInput")
    Wj_d = nc.dram_tensor("Wj", [2 * D, V], BF16, kind="ExternalInput")
    bj_d = nc.dram_tensor("bj", [V], BF16, kind="ExternalInput")
    out_d = nc.dram_tensor("out", [TC * U, V], FP32, kind="ExternalOutput")

    with tile.TileContext(nc) as tc:
        with (
            tc.tile_pool(name="const", bufs=1) as cp,
            tc.tile_pool(name="wts", bufs=1) as wp,
        ):
            # ---- constants ----
            # Output tile k (rows 128k..128k+128 of out, r = 64*jh + jl)
            # needs psO[r,v] = A[2k+jh, v] + C[jl, v].  With h = k//32,
            # m = k%32 the A row sits at offset 2m+jh inside A-half h.
            # selNC0 (h=0) selects from ACp0 = [A(0:64) ; C]:
            #   p<64:  1 iff p == 2m+jh      (A row)
            #   p>=64: 1 iff p-64 == jl      (C row)
            # selNC1 (h=1) selects from ACp1 = [C ; A(64:128)]: mirrored.
            selNC0 = cp.tile([128, 32 * 128], BF16, tag="selNC0")
            selNC1 = cp.tile([128, 32 * 128], BF16, tag="selNC1")
            A_PAT = [[-2, 32], [-1, 2], [0, 64]]  # p' - 2m - jh
            C_PAT = [[0, 32], [0, 2], [-1, 64]]  # p' - jl
            for tile_, pats in ((selNC0, (A_PAT, C_PAT)), (selNC1, (C_PAT, A_PAT))):
                nc.gpsimd.memset(tile_[:], 0.0)
                for s, pat in enumerate(pats):
                    sl = tile_[64 * s : 64 * s + 64, :]
                    nc.gpsimd.affine_select(
                        out=sl.rearrange("p (m a b) -> p m a b", m=32, a=2),
                        in_=sl.rearrange("p (m a b) -> p m a b", m=32, a=2),
                        compare_op=mybir.AluOpType.not_equal,
                        fill=1.0,
                        base=0,
                        pattern=pat,
                        channel_multiplier=1,
                    )

            ones2 = cp.tile([1, 128], BF16, tag="ones2")
            nc.gpsimd.memset(ones2[:], 1.0)

            # ---- persistent operands ----
            fT = [wp.tile([128, TC], BF16, tag=f"fT{c}", name=f"fT{c}") for c in range(4)]
            gT = [wp.tile([128, U], BF16, tag=f"gT{c}", name=f"gT{c}") for c in range(4)]
            We_sb = [wp.tile([128, D], BF16, tag=f"We{c}", name=f"We{c}") for c in range(4)]
            Wd_sb = [wp.tile([128, D], BF16, tag=f"Wd{c}", name=f"Wd{c}") for c in range(4)]
            Wj_sb = [wp.tile([128, V], BF16, tag=f"Wj{c}", name=f"Wj{c}") for c in range(8)]
            be_sb = [wp.tile([128, 1], FP32, tag=f"be{c}", name=f"be{c}") for c in range(4)]
            bd_sb = [wp.tile([128, 1], FP32, tag=f"bd{c}", name=f"bd{c}") for c in range(4)]
            bj_sb = wp.tile([1, V], BF16, tag="bj")
            tfT = [wp.tile([128, TC], BF16, tag=f"tfT{c}", name=f"tfT{c}") for c in range(4)]
            tgT = [wp.tile([128, U], BF16, tag=f"tgT{c}", name=f"tgT{c}") for c in range(4)]
            ACp0 = wp.tile([128, V], BF16, tag="ACp0")
            ACp1 = wp.tile([128, V], BF16, tag="ACp1")

            # input DMAs; Wj interleaved top/bottom so the A and C GEMMs
            # both stream as chunks land and neither owns the 9 us tail
            for c in range(4):
                nc.sync.dma_start(fT[c][:], fT_d[c * 128 : (c + 1) * 128, :])
                nc.sync.dma_start(We_sb[c][:], We_d[c * 128 : (c + 1) * 128, :])
                nc.sync.dma_start(
                    be_sb[c][:],
                    be_d[c * 128 : (c + 1) * 128].rearrange("(p o) -> p o", o=1),
                )
            for c in range(4):
                nc.sync.dma_start(gT[c][:], gT_d[c * 128 : (c + 1) * 128, :])
                nc.sync.dma_start(Wd_sb[c][:], Wd_d[c * 128 : (c + 1) * 128, :])
                nc.sync.dma_start(
                    bd_sb[c][:],
                    bd_d[c * 128 : (c + 1) * 128].rearrange("(p o) -> p o", o=1),
                )
            nc.sync.dma_start(bj_sb[:], bj_d.rearrange("(p v) -> p v", p=1))
            for c in (0, 4, 1, 5, 2, 6, 3, 7):
                nc.sync.dma_start(Wj_sb[c][:], Wj_d[c * 128 : (c + 1) * 128, :])

            # ---- prologue: tfT, tgT, then ACp0/ACp1 ----
            with tc.tile_pool(name="pp", bufs=4, space="PSUM") as pp:
                for mc in range(4):
                    ms = slice(mc * 128, (mc + 1) * 128)
                    ps = pp.tile([128, TC], FP32, tag="pps")
                    for dc in range(4):
                        nc.tensor.matmul(
                            ps[:],
                            We_sb[dc][:, ms],
                            fT[dc][:],
                            start=(dc == 0),
                            stop=(dc == 3),
                        )
                    nc.scalar.activation(
                        tfT[mc][:], ps[:], TANH, bias=be_sb[mc][:, 0:1]
                    )
                for mc in range(4):
                    ms = slice(mc * 128, (mc + 1) * 128)
                    ps = pp.tile([128, U], FP32, tag="pps")
                    for dc in range(4):
                        nc.tensor.matmul(
                            ps[:],
                            Wd_sb[dc][:, ms],
                            gT[dc][:],
                            start=(dc == 0),
                            stop=(dc == 3),
                        )
                    nc.scalar.activation(
                        tgT[mc][:], ps[:], TANH, bias=bd_sb[mc][:, 0:1]
                    )

                # A = tfT.T @ Wj_top -> ACp0[0:64], ACp1[64:128]
                for vh in range(2):
                    vs = slice(vh * 512, (vh + 1) * 512)
                    ps = pp.tile([128, 512], FP32, tag="pps")
                    for mc in range(4):
                        nc.tensor.matmul(
                            ps[:],
                            tfT[mc][:],
                            Wj_sb[mc][:, vs],
                            start=(mc == 0),
                            stop=(mc == 3),
                        )
                    nc.scalar.copy(ACp0[0:64, vs], ps[0:64, :])
                    nc.vector.tensor_copy(ACp1[64:128, vs], ps[64:128, :])

                # C = tgT.T @ Wj_bot + bj, written to both psum halves so
                # each ACp gets a same-partition copy
                for vh in range(2):
                    vs = slice(vh * 512, (vh + 1) * 512)
                    ps = pp.tile([128, 512], FP32, tag="pps")
                    for half in range(2):
                        hs = slice(half * 64, half * 64 + 64)
                        for mc in range(4):
                            nc.tensor.matmul(
                                ps[hs, :],
                                tgT[mc][:],
                                Wj_sb[4 + mc][:, vs],
                                start=(mc == 0),
                                stop=False,
                            )
                        nc.tensor.matmul(
                            ps[hs, :],
                            ones2[:, hs],
                            bj_sb[:, vs],
                            start=False,
                            stop=True,
                        )
                    nc.scalar.copy(ACp1[0:64, vs], ps[0:64, :])
                    nc.vector.tensor_copy(ACp0[64:128, vs], ps[64:128, :])

            # ---- main loop: 32 chunks of [256, 1024] = 1 MB each ----
            with (
                tc.tile_pool(name="po", bufs=2, space="PSUM") as po,
                tc.tile_pool(name="ob", bufs=4) as ob,
            ):
                for j in range(32):
                    psO = po.tile([128, 2 * V], FP32, tag="psO")
                    out_sb = ob.tile([128, 2 * V], FP32, tag="out")
                    for a in range(2):
                        k = 2 * j + a
                        h, m = k // 32, k % 32
                        sel = (selNC0, selNC1)[h]
                        acp = (ACp0, ACp1)[h]
                        for vh in range(2):
                            off = a * V + vh * 512
                            nc.tensor.matmul(
                                psO[:, off : off + 512],
                                sel[:, m * 128 : (m + 1) * 128],
                                acp[:, vh * 512 : (vh + 1) * 512],
                                start=True,
                                stop=True,
                            )
                    if j % 2 == 0:
                        nc.scalar.copy(out_sb[:], psO[:])
                    else:
                        nc.vector.tensor_copy(out_sb[:], psO[:])
                    nc.sync.dma_start(
                        out_d[256 * j : 256 * (j + 1), :].rearrange(
                            "(a p) v -> p a v", a=2
                        ),
                        out_sb[:].rearrange("p (a v) -> p a v", a=2),
                    )

    nc.compile()
    return nc


def kernel(f, g, We, be, Wd, bd, Wj, bj):
    if "nc" not in _cache:
        _cache["nc"] = _build_nc()
    nc = _cache["nc"]

    f32 = lambda x: np.ascontiguousarray(np.asarray(x), dtype=np.float32)
    b16 = lambda x: np.ascontiguousarray(np.asarray(x, dtype=np.float32).astype(NPBF16))
    f, g = np.asarray(f, dtype=np.float32), np.asarray(g, dtype=np.float32)
    shared = {
        "We": b16(We), "be": f32(be), "Wd": b16(Wd), "bd": f32(bd),
        "Wj": b16(Wj), "bj": b16(bj),
    }
    in_maps = []
    for c in range(NCORES):
        b, th = c // 2, c % 2
        in_maps.append(
            {
                "fT_c": b16(f[b, th * TC : (th + 1) * TC, :].T),
                "gT_c": b16(g[b].T),
                **shared,
            }
        )
    res = run_bass_kernel_spmd(nc, in_maps, list(range(NCORES)))
    kernel._last_results = res

    out = np.empty((B, T, U, V), np.float32)
    for c in range(NCORES):
        b, th = c // 2, c % 2
        out[b, th * TC : (th + 1) * TC] = res.results[c]["out"].reshape(TC, U, V)
    return out


# revision 4
# speedup vs baseline: 1.6720x; 1.5188x over previous
"""RNN-T JointNetwork kernel for 8 Trainium2 NeuronCores.

Math: out[b,t,u,:] = tanh(concat(fe[b,t], gd[b,u])) @ Wj + bj
with fe = f@We+be, gd = g@Wd+bd.

Since tanh acts elementwise and the concat feeds a single GEMM, the joint
GEMM factorizes exactly:
    out[b,t,u,:] = A[b,t,:] + C[b,u,:]
    A = tanh(f@We+be) @ Wj[:Dm]          (per-(b,t) row)
    C = tanh(g@Wd+bd) @ Wj[Dm:] + bj     (per-(b,u) row)
This collapses the 137-GFLOP joint GEMM into two tiny GEMMs plus a
broadcast-add, leaving the kernel bound by the output write.

Sharding: 8 cores, core c owns (b = c//2, t-half = c%2) -> a [128,64,V]
output chunk per core.

Key measured facts this design is built around (profiled on trn2):
  - each dma_start costs ~650 ns of *serial issue time* on the Sync
    sequencer -> pack inputs into 9 DMAs instead of 25
  - gpsimd affine_select is ~5 us per op -> ship all selector constants
    from the host inside the input pack
  - DMA write bandwidth measures ~400 GB/s -> the fp32 output write
    dominates everything; writing bf16 (tolerance 2e-2 >> bf16's ~5e-3)
    halves the bound, host upcasts to fp32 during unshard
  - fp32 PSUM->SBUF moves run at 1x on DVE and ACT alike -> alternate
    the per-tile output moves across both engines

On-core plan (bf16 everywhere, fp32 only in PSUM):
  - tfT[m,t] = tanh(We.T@fT + be), tgT likewise (PE bf16 + ACT tanh)
  - ACp0 = [A(0:64) ; C] and ACp1 = [C ; A(64:128)] packed bf16 [128,V]
    (C carries bj, added via K=1 ones-row matmuls into both psum halves)
  - per output tile k (t-pair 2k,2k+1): ONE K=128 selector matmul per
    512-col bank picks the A row and the C row and sums them in fp32
    PSUM; PSUM->SBUF bf16 copy alternates DVE/ACT; 512 KB DMA per tile
    pair
"""

import sys

sys.path.insert(0, "/opt/trn_rl_repo")

import numpy as np

import concourse.bacc as bacc
import concourse.mybir as mybir
import concourse.tile as tile
from concourse.bass_utils import run_bass_kernel_spmd

B, T, U = 4, 256, 64
D = 512  # DE = DD = DM
V = 1024
TC = 128  # t rows per core
NCORES = 8
FP32 = mybir.dt.float32
BF16 = mybir.dt.bfloat16
NPBF16 = mybir.dt.np(mybir.dt.bfloat16)
TANH = mybir.ActivationFunctionType.Tanh

# pack1 column offsets (per-core tensor: fT | We | gT | Wd)
OFF_FT, OFF_WE, OFF_GT, OFF_WD = 0, 512, 2560, 2816
PACK1_COLS = 4864

_cache = {}


def _build_nc():
    nc = bacc.Bacc("TRN2", target_bir_lowering=False)

    pack1_d = nc.dram_tensor("pack1", [128, PACK1_COLS], BF16, kind="ExternalInput")
    wjt_d = [nc.dram_tensor(f"wjt{i}", [128, 2048], BF16, kind="ExternalInput") for i in range(2)]
    wjb_d = [nc.dram_tensor(f"wjb{i}", [128, 2048], BF16, kind="ExternalInput") for i in range(2)]
    sel_d = [nc.dram_tensor(f"sel{i}", [128, 4096], BF16, kind="ExternalInput") for i in range(2)]
    bpack_d = nc.dram_tensor("bpack", [128, 8], FP32, kind="ExternalInput")
    bjp_d = nc.dram_tensor("bjp", [1, V + 128], BF16, kind="ExternalInput")
    out_d = nc.dram_tensor("out", [TC * U, V], BF16, kind="ExternalOutput")

    with tile.TileContext(nc) as tc:
        with tc.tile_pool(name="wts", bufs=1) as wp:
            pack1 = wp.tile([128, PACK1_COLS], BF16, tag="pack1")
            wjt = [wp.tile([128, 2048], BF16, tag=f"wjt{i}", name=f"wjt{i}") for i in range(2)]
            wjb = [wp.tile([128, 2048], BF16, tag=f"wjb{i}", name=f"wjb{i}") for i in range(2)]
            sel = [wp.tile([128, 4096], BF16, tag=f"sel{i}", name=f"sel{i}") for i in range(2)]
            bpack = wp.tile([128, 8], FP32, tag="bpack")
            bjp = wp.tile([1, V + 128], BF16, tag="bjp")
            tfT = [wp.tile([128, TC], BF16, tag=f"tfT{c}", name=f"tfT{c}") for c in range(4)]
            tgT = [wp.tile([128, U], BF16, tag=f"tgT{c}", name=f"tgT{c}") for c in range(4)]
            ACp0 = wp.tile([128, V], BF16, tag="ACp0")
            ACp1 = wp.tile([128, V], BF16, tag="ACp1")

            nc.sync.dma_start(bpack[:], bpack_d[:])
            nc.sync.dma_start(bjp[:], bjp_d[:])
            nc.sync.dma_start(pack1[:], pack1_d[:])
            for i in range(2):
                nc.sync.dma_start(wjb[i][:], wjb_d[i][:])
            for i in range(2):
                nc.sync.dma_start(wjt[i][:], wjt_d[i][:])
            for i in range(2):
                nc.sync.dma_start(sel[i][:], sel_d[i][:])

            # views into pack1
            fT = [pack1[:, OFF_FT + c * 128 : OFF_FT + (c + 1) * 128] for c in range(4)]
            We = [pack1[:, OFF_WE + c * 512 : OFF_WE + (c + 1) * 512] for c in range(4)]
            gT = [pack1[:, OFF_GT + c * 64 : OFF_GT + (c + 1) * 64] for c in range(4)]
            Wd = [pack1[:, OFF_WD + c * 512 : OFF_WD + (c + 1) * 512] for c in range(4)]
            # wj chunk mc, v-half vh  ->  tile mc//2, cols (mc%2)*1024 + vh*512
            wj_t = lambda mc, vh: wjt[mc // 2][:, (mc % 2) * 1024 + vh * 512 : (mc % 2) * 1024 + vh * 512 + 512]
            wj_b = lambda mc, vh: wjb[mc // 2][:, (mc % 2) * 1024 + vh * 512 : (mc % 2) * 1024 + vh * 512 + 512]

            # ---- prologue: tfT, tgT, then ACp0/ACp1 ----
            with tc.tile_pool(name="pp", bufs=4, space="PSUM") as pp:
                for mc in range(4):
                    ms = slice(mc * 128, (mc + 1) * 128)
                    ps = pp.tile([128, TC], FP32, tag="pps")
                    for dc in range(4):
                        nc.tensor.matmul(
                            ps[:], We[dc][:, ms], fT[dc],
                            start=(dc == 0), stop=(dc == 3),
                        )
                    nc.scalar.activation(
                        tfT[mc][:], ps[:], TANH, bias=bpack[:, mc : mc + 1]
                    )
                for mc in range(4):
                    ms = slice(mc * 128, (mc + 1) * 128)
                    ps = pp.tile([128, U], FP32, tag="pps")
                    for dc in range(4):
                        nc.tensor.matmul(
                            ps[:], Wd[dc][:, ms], gT[dc],
                            start=(dc == 0), stop=(dc == 3),
                        )
                    nc.scalar.activation(
                        tgT[mc][:], ps[:], TANH, bias=bpack[:, 4 + mc : 5 + mc]
                    )

                # C = tgT.T @ Wj_bot + bj, into both psum halves so each
                # ACp gets a same-partition copy
                for vh in range(2):
                    vs = slice(vh * 512, (vh + 1) * 512)
                    ps = pp.tile([128, 512], FP32, tag="pps")
                    for half in range(2):
                        hs = slice(half * 64, half * 64 + 64)
                        for mc in range(4):
                            nc.tensor.matmul(
                                ps[hs, :], tgT[mc][:], wj_b(mc, vh),
                                start=(mc == 0), stop=False,
                            )
                        nc.tensor.matmul(
                            ps[hs, :],
                            bjp[:, V + 64 * half : V + 64 * half + 64],
                            bjp[:, vh * 512 : (vh + 1) * 512],
                            start=False, stop=True,
                        )
                    nc.scalar.copy(ACp1[0:64, vs], ps[0:64, :])
                    nc.vector.tensor_copy(ACp0[64:128, vs], ps[64:128, :])

                # A = tfT.T @ Wj_top -> ACp0[0:64], ACp1[64:128]
                for vh in range(2):
                    vs = slice(vh * 512, (vh + 1) * 512)
                    ps = pp.tile([128, 512], FP32, tag="pps")
                    for mc in range(4):
                        nc.tensor.matmul(
                            ps[:], tfT[mc][:], wj_t(mc, vh),
                            start=(mc == 0), stop=(mc == 3),
                        )
                    nc.scalar.copy(ACp0[0:64, vs], ps[0:64, :])
                    nc.vector.tensor_copy(ACp1[64:128, vs], ps[64:128, :])

            # ---- main loop: 32 chunks of [256, 1024] bf16 = 512 KB ----
            with (
                tc.tile_pool(name="po", bufs=4, space="PSUM") as po,
                tc.tile_pool(name="ob", bufs=4) as ob,
            ):
                for j in range(32):
                    out_sb = ob.tile([128, 2 * V], BF16, tag="out")
                    for a in range(2):
                        k = 2 * j + a
                        h, m = k // 32, k % 32
                        psO = po.tile([128, V], FP32, tag="psO")
                        acp = (ACp0, ACp1)[h]
                        for vh in range(2):
                            nc.tensor.matmul(
                                psO[:, vh * 512 : (vh + 1) * 512],
                                sel[h][:, m * 128 : (m + 1) * 128],
                                acp[:, vh * 512 : (vh + 1) * 512],
                                start=True, stop=True,
                            )
                        dst = out_sb[:, a * V : (a + 1) * V]
                        if a == 0:
                            nc.scalar.copy(dst, psO[:])
                        else:
                            nc.vector.tensor_copy(dst, psO[:])
                    nc.sync.dma_start(
                        out_d[256 * j : 256 * (j + 1), :].rearrange(
                            "(a p) v -> p a v", a=2
                        ),
                        out_sb[:].rearrange("p (a v) -> p a v", a=2),
                    )

    nc.compile()
    return nc


def _chunkcat(M):
    """[N*128, C] -> [128, N*C]: stack 128-row chunks side by side."""
    n = M.shape[0] // 128
    return np.ascontiguousarray(
        M.reshape(n, 128, M.shape[1]).transpose(1, 0, 2).reshape(128, -1)
    )


def _build_selectors():
    """sel0/sel1: [128, 32*128] one-hot pair selectors (see module doc)."""
    sel0 = np.zeros((128, 4096), np.float32)
    sel1 = np.zeros((128, 4096), np.float32)
    jl = np.arange(64)
    for m in range(32):
        for jh in range(2):
            base = m * 128 + 64 * jh
            sel0[2 * m + jh, base : base + 64] = 1.0  # A row (half 0)
            sel0[64 + jl, base + jl] = 1.0  # C row
            sel1[jl, base + jl] = 1.0  # C row
            sel1[64 + 2 * m + jh, base : base + 64] = 1.0  # A row (half 1)
    return sel0.astype(NPBF16), sel1.astype(NPBF16)


def kernel(f, g, We, be, Wd, bd, Wj, bj):
    if "nc" not in _cache:
        _cache["nc"] = _build_nc()
    nc = _cache["nc"]

    b16 = lambda x: np.asarray(x, dtype=np.float32).astype(NPBF16)
    f = np.asarray(f, dtype=np.float32)
    g = np.asarray(g, dtype=np.float32)
    Wj = np.asarray(Wj, dtype=np.float32)

    sel0, sel1 = _build_selectors()
    wjt = _chunkcat(b16(Wj[:D]))
    wjb = _chunkcat(b16(Wj[D:]))
    bjp = np.zeros((1, V + 128), np.float32)
    bjp[0, :V] = np.asarray(bj, dtype=np.float32)
    bjp[0, V:] = 1.0
    bpack = np.zeros((128, 8), np.float32)
    for c in range(4):
        bpack[:, c] = np.asarray(be, dtype=np.float32)[c * 128 : (c + 1) * 128]
        bpack[:, 4 + c] = np.asarray(bd, dtype=np.float32)[c * 128 : (c + 1) * 128]
    We_p = _chunkcat(b16(We))
    Wd_p = _chunkcat(b16(Wd))

    shared = {
        "wjt0": np.ascontiguousarray(wjt[:, :2048]),
        "wjt1": np.ascontiguousarray(wjt[:, 2048:]),
        "wjb0": np.ascontiguousarray(wjb[:, :2048]),
        "wjb1": np.ascontiguousarray(wjb[:, 2048:]),
        "sel0": sel0, "sel1": sel1,
        "bpack": bpack, "bjp": b16(bjp),
    }
    in_maps = []
    for c in range(NCORES):
        b, th = c // 2, c % 2
        fTp = _chunkcat(b16(f[b, th * TC : (th + 1) * TC, :].T))
        gTp = _chunkcat(b16(g[b].T))
        pack1 = np.concatenate([fTp, We_p, gTp, Wd_p], axis=1)
        in_maps.append({"pack1": np.ascontiguousarray(pack1), **shared})
    res = run_bass_kernel_spmd(nc, in_maps, list(range(NCORES)))
    kernel._last_results = res

    out = np.empty((B, T, U, V), np.float32)
    for c in range(NCORES):
        b, th = c // 2, c % 2
        out[b, th * TC : (th + 1) * TC] = (
            res.results[c]["out"].astype(np.float32).reshape(TC, U, V)
        )
    return out


# revision 8
# speedup vs baseline: 1.7549x; 1.0496x over previous
"""RNN-T JointNetwork kernel for 8 Trainium2 NeuronCores.

Math: out[b,t,u,:] = tanh(concat(fe[b,t], gd[b,u])) @ Wj + bj
with fe = f@We+be, gd = g@Wd+bd.

Since tanh acts elementwise and the concat feeds a single GEMM, the joint
GEMM factorizes exactly:
    out[b,t,u,:] = A[b,t,:] + C[b,u,:]
    A = tanh(f@We+be) @ Wj[:Dm]          (per-(b,t) row)
    C = tanh(g@Wd+bd) @ Wj[Dm:] + bj     (per-(b,u) row)
This collapses the 137-GFLOP joint GEMM into two tiny GEMMs plus a
broadcast-add, leaving the kernel bound by the output write.

Sharding: 8 cores, core c owns (b = c//2, t-half = c%2) -> a [128,64,V]
output chunk per core.

Key measured facts this design is built around (profiled on trn2):
  - each dma_start costs ~650 ns of *serial issue time* on the Sync
    sequencer -> pack inputs into 9 DMAs instead of 25
  - gpsimd affine_select is ~5 us per op -> ship all selector constants
    from the host inside the input pack
  - DMA write bandwidth measures ~400 GB/s -> the fp32 output write
    dominates everything; writing bf16 (tolerance 2e-2 >> bf16's ~5e-3)
    halves the bound, host upcasts to fp32 during unshard
  - fp32 PSUM->SBUF moves run at 1x on DVE and ACT alike -> alternate
    the per-tile output moves across both engines

On-core plan (bf16 everywhere, fp32 only in PSUM):
  - tfT[m,t] = tanh(We.T@fT + be), tgT likewise (PE bf16 + ACT tanh)
  - ACp0 = [A(0:64) ; C] and ACp1 = [C ; A(64:128)] packed bf16 [128,V]
    (C carries bj, added via K=1 ones-row matmuls into both psum halves)
  - per output tile k (t-pair 2k,2k+1): ONE K=128 selector matmul per
    512-col bank picks the A row and the C row and sums them in fp32
    PSUM; PSUM->SBUF bf16 copy alternates DVE/ACT; 512 KB DMA per tile
    pair
"""

import sys

sys.path.insert(0, "/opt/trn_rl_repo")

import numpy as np

import concourse.bacc as bacc
import concourse.mybir as mybir
import concourse.tile as tile
from concourse.bass_utils import run_bass_kernel_spmd

B, T, U = 4, 256, 64
D = 512  # DE = DD = DM
V = 1024
TC = 128  # t rows per core
NCORES = 8
FP32 = mybir.dt.float32
BF16 = mybir.dt.bfloat16
NPBF16 = mybir.dt.np(mybir.dt.bfloat16)
TANH = mybir.ActivationFunctionType.Tanh

# pack1 column offsets (per-core tensor: fT | We | gT | Wd)
OFF_FT, OFF_WE, OFF_GT, OFF_WD = 0, 512, 2560, 2816
PACK1_COLS = 4864

_cache = {}


def _build_nc():
    nc = bacc.Bacc("TRN2", target_bir_lowering=False)

    pack1_d = nc.dram_tensor("pack1", [128, PACK1_COLS], BF16, kind="ExternalInput")
    wjt_d = [nc.dram_tensor(f"wjt{i}", [128, 2048], BF16, kind="ExternalInput") for i in range(2)]
    wjb_d = [nc.dram_tensor(f"wjb{i}", [128, 2048], BF16, kind="ExternalInput") for i in range(2)]
    sel_d = [nc.dram_tensor(f"sel{i}", [128, 4096], BF16, kind="ExternalInput") for i in range(2)]
    bpack_d = nc.dram_tensor("bpack", [128, 8], FP32, kind="ExternalInput")
    bjp_d = nc.dram_tensor("bjp", [1, V + 128], BF16, kind="ExternalInput")
    out_d = nc.dram_tensor("out", [TC * U, V], BF16, kind="ExternalOutput")

    with tile.TileContext(nc) as tc:
        with tc.tile_pool(name="wts", bufs=1) as wp:
            pack1 = wp.tile([128, PACK1_COLS], BF16, tag="pack1")
            wjt = [wp.tile([128, 2048], BF16, tag=f"wjt{i}", name=f"wjt{i}") for i in range(2)]
            wjb = [wp.tile([128, 2048], BF16, tag=f"wjb{i}", name=f"wjb{i}") for i in range(2)]
            sel = [wp.tile([128, 4096], BF16, tag=f"sel{i}", name=f"sel{i}") for i in range(2)]
            bpack = wp.tile([128, 8], FP32, tag="bpack")
            bjp = wp.tile([1, V + 128], BF16, tag="bjp")
            tfT = [wp.tile([128, TC], BF16, tag=f"tfT{c}", name=f"tfT{c}") for c in range(4)]
            tgT = [wp.tile([128, U], BF16, tag=f"tgT{c}", name=f"tgT{c}") for c in range(4)]
            ACp0 = wp.tile([128, V], BF16, tag="ACp0")
            ACp1 = wp.tile([128, V], BF16, tag="ACp1")

            nc.sync.dma_start(pack1[:], pack1_d[:])
            nc.sync.dma_start(bpack[:], bpack_d[:])
            nc.sync.dma_start(bjp[:], bjp_d[:])
            for i in range(2):
                nc.sync.dma_start(wjb[i][:], wjb_d[i][:])
            for i in range(2):
                nc.sync.dma_start(wjt[i][:], wjt_d[i][:])
            for i in range(2):
                nc.sync.dma_start(sel[i][:], sel_d[i][:])

            # views into pack1
            fT = [pack1[:, OFF_FT + c * 128 : OFF_FT + (c + 1) * 128] for c in range(4)]
            We = [pack1[:, OFF_WE + c * 512 : OFF_WE + (c + 1) * 512] for c in range(4)]
            gT = [pack1[:, OFF_GT + c * 64 : OFF_GT + (c + 1) * 64] for c in range(4)]
            Wd = [pack1[:, OFF_WD + c * 512 : OFF_WD + (c + 1) * 512] for c in range(4)]
            # wj chunk mc, v-half vh  ->  tile mc//2, cols (mc%2)*1024 + vh*512
            wj_t = lambda mc, vh: wjt[mc // 2][:, (mc % 2) * 1024 + vh * 512 : (mc % 2) * 1024 + vh * 512 + 512]
            wj_b = lambda mc, vh: wjb[mc // 2][:, (mc % 2) * 1024 + vh * 512 : (mc % 2) * 1024 + vh * 512 + 512]

            # ---- prologue: tfT, tgT, then ACp0/ACp1 ----
            with tc.tile_pool(name="pp", bufs=4, space="PSUM") as pp:
                # PE warm-up: ~20 dummy matmuls on an uninitialized scratch
                # tile run while the input DMAs stream, so the HAM clock
                # gate reaches 8/8 before the real GEMMs (and stays there
                # through the main loop).  Results are never read.
                scratch = wp.tile([128, 640], BF16, tag="scratch")
                nc.vector.memset(scratch[:], 1.0)
                wps = pp.tile([128, 512], FP32, tag="warm")
                for _ in range(20):
                    nc.tensor.matmul(
                        wps[:], scratch[:, 0:128], scratch[:, 128:640],
                        start=True, stop=True,
                    )
                for mc in range(4):
                    ms = slice(mc * 128, (mc + 1) * 128)
                    ps = pp.tile([128, TC], FP32, tag="pps")
                    for dc in range(4):
                        nc.tensor.matmul(
                            ps[:], We[dc][:, ms], fT[dc],
                            start=(dc == 0), stop=(dc == 3),
                        )
                    nc.scalar.activation(
                        tfT[mc][:], ps[:], TANH, bias=bpack[:, mc : mc + 1]
                    )
                for mc in range(4):
                    ms = slice(mc * 128, (mc + 1) * 128)
                    ps = pp.tile([128, U], FP32, tag="pps")
                    for dc in range(4):
                        nc.tensor.matmul(
                            ps[:], Wd[dc][:, ms], gT[dc],
                            start=(dc == 0), stop=(dc == 3),
                        )
                    nc.scalar.activation(
                        tgT[mc][:], ps[:], TANH, bias=bpack[:, 4 + mc : 5 + mc]
                    )

                # C = tgT.T @ Wj_bot + bj, into both psum halves so each
                # ACp gets a same-partition copy
                for vh in range(2):
                    vs = slice(vh * 512, (vh + 1) * 512)
                    ps = pp.tile([128, 512], FP32, tag="pps")
                    for half in range(2):
                        hs = slice(half * 64, half * 64 + 64)
                        for mc in range(4):
                            nc.tensor.matmul(
                                ps[hs, :], tgT[mc][:], wj_b(mc, vh),
                                start=(mc == 0), stop=False,
                            )
                        nc.tensor.matmul(
                            ps[hs, :],
                            bjp[:, V + 64 * half : V + 64 * half + 64],
                            bjp[:, vh * 512 : (vh + 1) * 512],
                            start=False, stop=True,
                        )
                    nc.scalar.copy(ACp1[0:64, vs], ps[0:64, :])
                    nc.vector.tensor_copy(ACp0[64:128, vs], ps[64:128, :])

                # A = tfT.T @ Wj_top -> ACp0[0:64], ACp1[64:128]
                for vh in range(2):
                    vs = slice(vh * 512, (vh + 1) * 512)
                    ps = pp.tile([128, 512], FP32, tag="pps")
                    for mc in range(4):
                        nc.tensor.matmul(
                            ps[:], tfT[mc][:], wj_t(mc, vh),
                            start=(mc == 0), stop=(mc == 3),
                        )
                    nc.scalar.copy(ACp0[0:64, vs], ps[0:64, :])
                    nc.vector.tensor_copy(ACp1[64:128, vs], ps[64:128, :])

            # ---- main loop: 16 chunks of [512, 1024] bf16 = 1 MB ----
            with (
                tc.tile_pool(name="po", bufs=4, space="PSUM") as po,
                tc.tile_pool(name="ob", bufs=3) as ob,
            ):
                for j in range(16):
                    out_sb = ob.tile([128, 4 * V], BF16, tag="out")
                    for a in range(4):
                        k = 4 * j + a
                        h, m = k // 32, k % 32
                        psO = po.tile([128, V], FP32, tag="psO")
                        acp = (ACp0, ACp1)[h]
                        for vh in range(2):
                            nc.tensor.matmul(
                                psO[:, vh * 512 : (vh + 1) * 512],
                                sel[h][:, m * 128 : (m + 1) * 128],
                                acp[:, vh * 512 : (vh + 1) * 512],
                                start=True, stop=True,
                            )
                        dst = out_sb[:, a * V : (a + 1) * V]
                        if a % 2 == 0:
                            nc.scalar.copy(dst, psO[:])
                        else:
                            nc.vector.tensor_copy(dst, psO[:])
                    nc.sync.dma_start(
                        out_d[512 * j : 512 * (j + 1), :].rearrange(
                            "(a p) v -> p a v", a=4
                        ),
                        out_sb[:].rearrange("p (a v) -> p a v", a=4),
                    )

    nc.compile()
    return nc


def _chunkcat(M):
    """[N*128, C] -> [128, N*C]: stack 128-row chunks side by side."""
    n = M.shape[0] // 128
    return np.ascontiguousarray(
        M.reshape(n, 128, M.shape[1]).transpose(1, 0, 2).reshape(128, -1)
    )


def _build_selectors():
    """sel0/sel1: [128, 32*128] one-hot pair selectors (see module doc)."""
    sel0 = np.zeros((128, 4096), np.float32)
    sel1 = np.zeros((128, 4096), np.float32)
    jl = np.arange(64)
    for m in range(32):
        for jh in range(2):
            base = m * 128 + 64 * jh
            sel0[2 * m + jh, base : base + 64] = 1.0  # A row (half 0)
            sel0[64 + jl, base + jl] = 1.0  # C row
            sel1[jl, base + jl] = 1.0  # C row
            sel1[64 + 2 * m + jh, base : base + 64] = 1.0  # A row (half 1)
    return sel0.astype(NPBF16), sel1.astype(NPBF16)


def kernel(f, g, We, be, Wd, bd, Wj, bj):
    if "nc" not in _cache:
        _cache["nc"] = _build_nc()
    nc = _cache["nc"]

    b16 = lambda x: np.asarray(x, dtype=np.float32).astype(NPBF16)
    f = np.asarray(f, dtype=np.float32)
    g = np.asarray(g, dtype=np.float32)
    Wj = np.asarray(Wj, dtype=np.float32)

    sel0, sel1 = _build_selectors()
    wjt = _chunkcat(b16(Wj[:D]))
    wjb = _chunkcat(b16(Wj[D:]))
    bjp = np.zeros((1, V + 128), np.float32)
    bjp[0, :V] = np.asarray(bj, dtype=np.float32)
    bjp[0, V:] = 1.0
    bpack = np.zeros((128, 8), np.float32)
    for c in range(4):
        bpack[:, c] = np.asarray(be, dtype=np.float32)[c * 128 : (c + 1) * 128]
        bpack[:, 4 + c] = np.asarray(bd, dtype=np.float32)[c * 128 : (c + 1) * 128]
    We_p = _chunkcat(b16(We))
    Wd_p = _chunkcat(b16(Wd))

    shared = {
        "wjt0": np.ascontiguousarray(wjt[:, :2048]),
        "wjt1": np.ascontiguousarray(wjt[:, 2048:]),
        "wjb0": np.ascontiguousarray(wjb[:, :2048]),
        "wjb1": np.ascontiguousarray(wjb[:, 2048:]),
        "sel0": sel0, "sel1": sel1,
        "bpack": bpack, "bjp": b16(bjp),
    }
    in_maps = []
    for c in range(NCORES):
        b, th = c // 2, c % 2
        fTp = _chunkcat(b16(f[b, th * TC : (th + 1) * TC, :].T))
        gTp = _chunkcat(b16(g[b].T))
        pack1 = np.concatenate([fTp, We_p, gTp, Wd_p], axis=1)
        in_maps.append({"pack1": np.ascontiguousarray(pack1), **shared})
    res = run_bass_kernel_spmd(nc, in_maps, list(range(NCORES)))
    kernel._last_results = res

    out = np.empty((B, T, U, V), np.float32)
    for c in range(NCORES):
        b, th = c // 2, c % 2
        out[b, th * TC : (th + 1) * TC] = (
            res.results[c]["out"].astype(np.float32).reshape(TC, U, V)
        )
    return out
